# revision 18
# baseline (speedup 1.0000x reference)
# Trainium2 Bass kernel for nn_Decoder_14568529068506 (gnn_message_passing).
#
# Reference computation (per scene s of 32, P=48 peds):
#   rel[i,j]  = obs[j] - obs[i]                  (P,P,2T)   2T=16
#   emb       = rel @ W_se.T                     (P,P,512)
#   emb      *= tile(traj_weight[s])             (P,P,512)
#   x         = concat([emb, h[j]], -1)          (P,P,576)
#   x1        = relu(x @ W1.T + b1)              (P,P,512)
#   x2        = relu(x1 @ W2.T + b2)             (P,P,1024)
#   out[s,i]  = max_j x2[i,j]                    (P,1024)
#
# Kernel restructuring (validated in fp32 numpy):
#  * traj_weight tiling + spatial embedding + W1 fused on the host:
#      out1[d,row] = sum_{(ct,g)} Wf[d,(ct,g)] * tw[row,ct] * rel[row,g]
#    so MLP1 contracts over 256 "rel2" features; the (P,P,512) embedding
#    never exists.
#  * The h-state part of MLP1 is column-constant within a scene:
#    y_h = W1h @ h + b1 is computed once per scene as a tiny N=48 matmul
#    (bias via a constant-1 input row) and injected per block by one DVE
#    add, so the per-pair MLP1 is two K=128 matmuls per m-tile - the PE
#    stream has a single weight row-group config (no switch bubbles).
#  * All data replication (tw -> 128 partitions, obs -> (r,g)x(i,j)
#    layouts) is pure layout, done on the HOST and shipped as ONE packed
#    DMA per scene; the arithmetic rel = obsJ - obsI, rel2 = tw * rel
#    runs on GPSIMD (SBUF-only engine).
#  * relu/bias commute with max-pool; MLP2 outputs are max-pooled out of
#    PSUM by DVE (one reduce per 128-row m-tile); bias+relu run
#    post-pool on the Act engine into an f16 [128,8,48] tile, one output
#    DMA per scene.
#  * fp16 operands everywhere (1 cycle/row on PE, same speed as bf16,
#    8x the mantissa accuracy of bf16). PSUM accumulation stays fp32.
#  * The host does the final output transpose to (B, 1024) f32 -
#    layout only.
#
# Sharding: scenes are data-parallel across the 8 cores (4 scenes each);
# weights replicated; per-core outputs are concatenated on the host.

import numpy as np

S, P, T, E, H = 32, 48, 8, 64, 64
D1, D2 = 512, 1024
B = S * P
NCORES = 8
SC = S // NCORES          # scenes per core
NB = 6                    # row blocks per scene
NBLK = P * P // NB        # 384 columns (pairs) per block = 8 i-groups x 48 j
IB = NBLK // P            # i-groups per block (8)
PP = P * P                # 2304
# packed per-scene input: chunk-major [NB][obsI | obsJ | tw0 | tw1][NBLK]
# so both the whole-scene DMA and the per-chunk scene-0 DMAs are contiguous


def _host_constants(W_se, W1, W2, b1, b2):
    """Fused weights + lhsT layouts (fp32; cast to f16 in _host_inputs)."""
    W_se = np.asarray(W_se, np.float32)
    W1 = np.asarray(W1, np.float32)
    W2 = np.asarray(W2, np.float32)
    b1 = np.asarray(b1, np.float32)
    b2 = np.asarray(b2, np.float32)

    W1e, W1h = W1[:, :512], W1[:, 512:]
    Wf = np.zeros((D1, 256), np.float32)
    for c in range(2):
        for t in range(T):
            ct = c * 8 + t
            f = t * 64 + np.arange(c, 64, 2)
            Wf[:, ct * 16:(ct + 1) * 16] = W1e[:, f] @ W_se[f, :]

    # lhsT tile layouts: [K(128 part), kTiles, M]
    Wf_sb = np.ascontiguousarray(Wf.T.reshape(2, 128, D1).transpose(1, 0, 2))
    # W1h padded to K=128: row 64 is the constant-1 bias row carrying b1
    W1hp = np.zeros((128, D1), np.float32)
    W1hp[:64] = W1h.T
    W1hp[64] = b1
    W2_sb = np.ascontiguousarray(W2.T.reshape(4, 128, D2).transpose(1, 0, 2))
    b2_sb = np.ascontiguousarray(b2.reshape(8, 128).T)       # (128, 8)
    return dict(Wf_sb=Wf_sb, W1hp=W1hp, W2_sb=W2_sb, b2_sb=b2_sb)


def build_program(n_scenes=SC):
    """Emit the per-core Bass/Tile program. Returns the compiled Bacc."""
    from contextlib import ExitStack
    import concourse.bacc as bacc
    import concourse.tile as tile
    from concourse import mybir
    from concourse.alu_op_type import AluOpType

    f32 = mybir.dt.float32
    f16 = mybir.dt.float16
    AF = mybir.ActivationFunctionType
    AX = mybir.AxisListType

    nc = bacc.Bacc("TRN2", target_bir_lowering=False, debug=False)

    # ---- DRAM parameters -------------------------------------------------
    d_combo = nc.dram_tensor("combo", [n_scenes, 128, NB, 4, NBLK], f16, kind="ExternalInput")
    d_h = nc.dram_tensor("h_scp", [n_scenes, 128, P], f16, kind="ExternalInput")
    d_Wf = nc.dram_tensor("Wf_sb", [128, 2, D1], f16, kind="ExternalInput")
    d_W1hp = nc.dram_tensor("W1hp", [128, D1], f16, kind="ExternalInput")
    d_W2 = nc.dram_tensor("W2_sb", [128, 4, D2], f16, kind="ExternalInput")
    d_b2 = nc.dram_tensor("b2_sb", [128, 8], f32, kind="ExternalInput")
    d_out = nc.dram_tensor("out", [n_scenes, 128, 8, P], f16, kind="ExternalOutput")

    with ExitStack() as ctx:
        tc = ctx.enter_context(tile.TileContext(nc))
        consts = ctx.enter_context(tc.tile_pool(name="consts", bufs=1))
        scene_pool = ctx.enter_context(tc.tile_pool(name="scene", bufs=2))
        blk_pool = ctx.enter_context(tc.tile_pool(name="blk", bufs=3))
        p1 = ctx.enter_context(tc.tile_pool(name="p1", bufs=3, space="PSUM"))
        p2 = ctx.enter_context(tc.tile_pool(name="p2", bufs=4, space="PSUM"))
        pyh = ctx.enter_context(tc.tile_pool(name="pyh", bufs=1, space="PSUM"))

        # weight DMAs ordered for the startup critical path: W1hp (y_h of
        # scene 0) first; Wf (first MLP1) and W2 are interleaved with the
        # scene-0 chunk DMAs below, each just ahead of first use.
        scratch = consts.tile([128, P], f16)
        nc.vector.memset(scratch[:], 0.0)
        W1hp_sb = consts.tile([128, D1], f16)
        nc.sync.dma_start(W1hp_sb[:], d_W1hp[:])
        Wf_sb = consts.tile([128, 2, D1], f16)
        W2_sb = consts.tile([128, 4, D2], f16)
        b2_sb = consts.tile([128, 8], f32)

        blocks = [(s, b) for s in range(n_scenes) for b in range(NB)]
        state = {}   # per-scene tiles
        mlp_q = []   # software pipeline: deferred MLP2 stage

        def prep_chunk(eng, st, ch):
            combo, rel, rel2 = st["combo"], st["rel"], st["rel2"]
            cs = slice(ch * NBLK, (ch + 1) * NBLK)
            eng.tensor_tensor(rel[:, cs], combo[:, ch, 1, :],
                              combo[:, ch, 0, :], AluOpType.subtract)
            eng.tensor_tensor(rel2[:, 0, cs], rel[:, cs],
                              combo[:, ch, 2, :], AluOpType.mult)
            eng.tensor_tensor(rel2[:, 1, cs], rel[:, cs],
                              combo[:, ch, 3, :], AluOpType.mult)

        def new_scene(s):
            st = state[s] = dict(
                h_scp=scene_pool.tile([128, P], f16, tag="h_scp", name="h_scp"),
                combo=scene_pool.tile([128, NB, 4, NBLK], f16, tag="combo", name="combo"),
                rel=scene_pool.tile([128, PP], f16, tag="rel", name="rel"),
                rel2=scene_pool.tile([128, 2, PP], f16, tag="rel2", name="rel2"),
                pooled=scene_pool.tile([128, 8, P], f32, tag="pooled", name="pooled"),
                out_sb=scene_pool.tile([128, 8, P], f16, tag="out_sb", name="out_sb"))
            nc.sync.dma_start(st["h_scp"][:], d_h[s])
            return st

        def setup_dma(s):
            """DMA the scene's host-packed tiles; GPSIMD builds rel2."""
            st = new_scene(s)
            nc.sync.dma_start(st["combo"][:], d_combo[s])
            for ch in range(NB):
                prep_chunk(nc.gpsimd, st, ch)

        def setup_scene0():
            """Scene 0 arrives in contiguous block-aligned chunks; the
            first chunks' prep runs on DVE (fast, idle at start), the
            rest on GPSIMD. Wf/W2 weight DMAs are slotted where the
            startup critical path wants them."""
            st = new_scene(0)
            for ch in range(NB):
                nc.sync.dma_start(st["combo"][:, ch], d_combo[0][:, ch])
                prep_chunk(nc.vector if ch <= 1 else nc.gpsimd, st, ch)
                if ch == 0:
                    nc.sync.dma_start(Wf_sb[:], d_Wf[:])
                elif ch == 1:
                    nc.sync.dma_start(W2_sb[:], d_W2[:])

        def setup_yh(s, warmup=False):
            """y_h[m] = W1h @ h + b1 as four tiny N=48 matmuls."""
            st = state[s]
            yh_ps = pyh.tile([128, 4, P], f32, tag="yh")
            if warmup:
                # dummy matmuls on a memset tile (no DMA dependency) so
                # the PE p-state ramp (3us of continuous execution ->
                # 2.4GHz) completes while the startup DMAs are in flight;
                # sized to bridge until the first rel2 chunk is ready.
                for _ in range(160):
                    nc.tensor.matmul(yh_ps[:P, 0, :], scratch[:],
                                     scratch[:], start=True, stop=True)
            for m in range(4):
                nc.tensor.matmul(yh_ps[:, m, :],
                                 W1hp_sb[:, m * 128:(m + 1) * 128],
                                 st["h_scp"][:], start=True, stop=True)
            yh_sb = scene_pool.tile([128, 4, P], f32, tag="yh_sb")
            nc.scalar.copy(yh_sb[:], yh_ps[:])
            st["yh_sb"] = yh_sb

        def mlp1(s, b):
            st = state[s]
            c0 = b * NBLK
            x1 = blk_pool.tile([128, 4, NBLK], f16, tag="x1")
            for m in range(4):
                p1t = p1.tile([128, NBLK], f32, tag="p1")
                nc.tensor.matmul(p1t[:], Wf_sb[:, 0, m * 128:(m + 1) * 128],
                                 st["rel2"][:, 0, c0:c0 + NBLK],
                                 start=True, stop=False)
                nc.tensor.matmul(p1t[:], Wf_sb[:, 1, m * 128:(m + 1) * 128],
                                 st["rel2"][:, 1, c0:c0 + NBLK],
                                 start=False, stop=True)
                nc.vector.tensor_tensor(
                    x1[:, m, :].rearrange("p (i j) -> p i j", i=IB),
                    p1t[:].rearrange("p (i j) -> p i j", i=IB),
                    st["yh_sb"][:, m, :].unsqueeze(1).broadcast_to([128, IB, P]),
                    AluOpType.add)
                nc.scalar.activation(x1[:, m, :], x1[:, m, :], AF.Relu)
            return x1

        def mlp2(s, b, x1):
            st = state[s]
            for mm in range(8):
                p2t = p2.tile([128, 512], f32, tag="p2")
                for k in range(4):
                    nc.tensor.matmul(
                        p2t[:, :NBLK],
                        W2_sb[:, k, mm * 128:(mm + 1) * 128],
                        x1[:, k, :], start=(k == 0), stop=(k == 3))
                nc.vector.tensor_reduce(
                    st["pooled"][:, mm, b * IB:(b + 1) * IB],
                    p2t[:, :NBLK].rearrange("p (i j) -> p i j", i=IB),
                    axis=AX.X, op=AluOpType.max)
            if b == NB - 1:
                for mm in range(8):
                    nc.scalar.activation(
                        st["out_sb"][:, mm, :], st["pooled"][:, mm, :],
                        AF.Relu, bias=b2_sb[:, mm:mm + 1])
                nc.sync.dma_start(d_out[s], st["out_sb"][:])
                state.pop(s)

        # two-deep software pipeline on PE: ... mlp1(i)  mlp2(i-1) ...
        # scene DMAs+prep prefetched a full scene early (GPSIMD prep of a
        # scene takes ~34us); y_h matmuls land mid-scene so the PE never
        # waits on their input DMAs.
        setup_scene0()
        setup_yh(0, warmup=True)
        nc.sync.dma_start(b2_sb[:], d_b2[:])
        for idx, (s, b) in enumerate(blocks):
            if b == 0 and s + 1 < n_scenes:
                setup_dma(s + 1)
            if b == 3 and s + 1 < n_scenes:
                setup_yh(s + 1)
            mlp_q.append((s, b, mlp1(s, b)))
            if len(mlp_q) > 1:
                mlp2(*mlp_q.pop(0))
        mlp2(*mlp_q.pop(0))

    nc.compile()
    return nc


def _host_inputs(h_states, traj, traj_weight, consts, n_scenes=SC):
    """Slice + lay out per-core input maps (all matmul operands f16)."""
    f16 = np.float16
    h_states = np.asarray(h_states, np.float32)
    traj = np.asarray(traj, np.float32)
    traj_weight = np.asarray(traj_weight, np.float32)

    obs = traj[:T].transpose(1, 0, 2).reshape(S, P, 2 * T)   # (S,P,16) g=t*2+c
    h_full = h_states.reshape(S, P, H)

    # obsT[s, p=(r*16+g), j] = obs[s, j, g]   (replica r = 0..7)
    obsT = np.tile(obs.transpose(0, 2, 1), (1, 8, 1))        # (S,128,48)
    # twX[s, p=(r*16+g), col] = tw[s, ct, col], ct = r (tw0) / 8+r (tw1)
    twT = np.ascontiguousarray(
        traj_weight.transpose(0, 2, 3, 1).reshape(S, 16, PP))
    combo = np.empty((S, 128, NB, 4, NBLK), f16)
    cv = combo.reshape(S, 128, NB, 4, NBLK)
    cv[:, :, :, 0] = np.repeat(obsT, P, axis=2).reshape(S, 128, NB, NBLK)
    cv[:, :, :, 1] = np.tile(obsT, (1, 1, P)).reshape(S, 128, NB, NBLK)
    cv[:, :, :, 2] = np.repeat(twT[:, 0:8], 16, axis=1).reshape(S, 128, NB, NBLK)
    cv[:, :, :, 3] = np.repeat(twT[:, 8:16], 16, axis=1).reshape(S, 128, NB, NBLK)
    # h_scp[s, k, j] = h[s, j, k] padded to K=128 with the bias row at 64
    h_scp = np.zeros((S, 128, P), f16)
    h_scp[:, :64] = h_full.transpose(0, 2, 1)
    h_scp[:, 64] = 1.0

    c16 = {k: (v.astype(f16) if k in ("Wf_sb", "W1hp", "W2_sb") else v)
           for k, v in consts.items()}

    in_maps = []
    for core in range(NCORES):
        sl = slice(core * n_scenes, (core + 1) * n_scenes)
        m = dict(combo=np.ascontiguousarray(combo[sl]),
                 h_scp=np.ascontiguousarray(h_scp[sl]))
        m.update(c16)
        in_maps.append(m)
    return in_maps


def kernel(h_states, seq_start_end, end_pos, traj, traj_weight,
           mlp_pre_pool_dim_0, W_se, b_se, W1, b1, W2, b2):
    import sys
    if '/opt/trn_rl_repo' not in sys.path:
        sys.path.insert(0, '/opt/trn_rl_repo')
    from concourse.bass_utils import run_bass_kernel_spmd

    consts = _host_constants(W_se, W1, W2, b1, b2)
    in_maps = _host_inputs(h_states, traj, traj_weight, consts)
    nc = build_program(SC)
    res = run_bass_kernel_spmd(nc, in_maps, list(range(NCORES)))
    # device output: [n_scenes, 128 (d%128), 8 (d//128), 48 (i)] f16 per core
    parts = []
    for i in range(NCORES):
        o = np.asarray(res.results[i]["out"], np.float32)
        parts.append(o.transpose(0, 3, 2, 1).reshape(SC * P, D2))
    return np.concatenate(parts, axis=0)
